# revision 43
# baseline (speedup 1.0000x reference)
"""Causal self-attention (B=8, T=1024, C=768, NH=12) on 8 TRN2 NeuronCores.

Sharding: pure data-parallel over batch -- one batch element per core, weights
replicated. No collectives needed.

Per-core algorithm (v5):
  - x, w_attn, w_proj are cast to bf16 on the host (rel err ~4e-3 vs the 2e-2
    gate), halving startup HBM traffic.
  - x^T lands in SBUF directly via XBAR DMA-transpose (no PE transposes, no
    PSUM->SBUF evictions). DMA issue cost (~0.6us/transfer on the issuing
    queue) is spread across engines: sync carries x^T, scalar carries
    WV/WQK + the y writebacks, gpsimd (software DGE, ~25ns issue) carries
    biases + WP.
  - V projection per row tile (+bias scatter into the pair-grouped V_aug fp16
    layout with appended ones-columns).
  - QK projection per head-pair is WOVEN into the previous pair's attention
    so PE work hides ACT-exp latency. The K bias is dropped entirely (softmax
    over keys is invariant to it once Q carries its own bias): K eviction is
    a plain cast-copy, Q eviction adds b_q.
  - Attention per pair, query-halved (ci-split): O/denominator accumulate in
    2 PSUM banks per half, freeing banks for the woven projections.
    ST = K^T-block @ Q (fp16, keys on partitions), one exp per chunk (both
    heads share a 2-bank ST tile), causal mask as a 0/1 fp16 multiply on the
    diagonal 128-block, O^T += V_aug-block @ P with ones-columns producing
    the softmax denominator rows for free. P is fp16 (logits*scale ~
    N(0, 0.31)). Normalization: reciprocal from PSUM (base-0 partitions) or
    via one shift-copy; one multiply into bf16 OT.
  - y = OT.T @ Wp + bp. Row tiles 0-3 only need the first query-half of
    pair 5's OT, so they are woven into pair 5's second attention half.
"""
import numpy as np
from contextlib import ExitStack

import concourse.bass as bass
import concourse.tile as tile
from concourse import bacc, mybir
from concourse.bass_utils import run_bass_kernel_spmd
from concourse.masks import make_identity, make_upper_triangular

T, C, NH, HD = 1024, 768, 12, 64
N_CORES = 8
SCALE = 1.0 / 8.0  # 1/sqrt(HD)

F32 = mybir.dt.float32
F16 = mybir.dt.float16
BF16 = mybir.dt.bfloat16
U16 = mybir.dt.uint16
EXP = mybir.ActivationFunctionType.Exp
ADD = mybir.AluOpType.add
BYPASS = mybir.AluOpType.bypass


def _body(ctx, tc, y, x, w_attn, b_attn, w_proj, b_proj):
    nc = tc.nc

    const = ctx.enter_context(tc.tile_pool(name="const", bufs=1))
    # PSUM: st 2x[128,1024]f32 (4 banks: ST tiles), ot 2x[128,512]f32 (2
    # banks: O/denominator, one query-half at a time), pq 2x[128,512]f32 (2
    # banks: V/QK/out projections). 16KB/partition total.
    psum = ctx.enter_context(tc.tile_pool(name="psum", bufs=2, space="PSUM"))

    # ---- constants (gpsimd; also used to warm the PE p-state) ----
    ident = const.tile([128, 128], BF16, tag="ident", name="ident")
    make_identity(nc, ident[:])
    # mask[j, i] = 1 if i >= j else 0 (keep key j for query i when i >= j)
    mask = const.tile([128, 128], F16, tag="mask", name="mask")
    make_upper_triangular(nc, mask[:], val=1.0, diag=True)

    # ---- persistent tensors ----
    qk_pool = ctx.enter_context(tc.tile_pool(name="qk", bufs=1))
    QT = [qk_pool.tile([128, T], F16, tag=f"qt{i}", name=f"qt{i}") for i in range(6)]
    KT = [qk_pool.tile([128, T], F16, tag=f"kt{i}", name=f"kt{i}") for i in range(6)]
    # V_aug: 6 pair-groups of 192 cols: [V_{2p} (64) | ones (64) | V_{2p+1} (64)]
    # -> per-head lhsT is the contiguous 128-col slice [p*192 + (h%2)*64, +128):
    #    even head: [V_h | ones] (PSUM rows 0:64 = O^T, 64:128 = denom)
    #    odd head:  [ones | V_h] (rows flipped)
    v_pool = ctx.enter_context(tc.tile_pool(name="v", bufs=1))
    V = [v_pool.tile([128, 1152], F16, tag=f"v{i}", name=f"v{i}") for i in range(8)]

    xw_pool = ctx.enter_context(tc.tile_pool(name="xw", bufs=1))
    # XT[c_part, kc*1024 + t]: transposed x, channel-major
    XT = xw_pool.tile([128, 6 * T], BF16, tag="xt", name="xt")
    # WQKp[pr][c_part, kc*256 + which*128 + d]: per-pair column slices of
    # w_attn, so pair pr's weights stream in just before its woven projection
    WQKp = [xw_pool.tile([128, 1536], BF16, tag=f"w{i}", name=f"w{i}")
            for i in range(6)]
    ot_pool = ctx.enter_context(tc.tile_pool(name="ot", bufs=1))
    OT = [ot_pool.tile([128, T], BF16, tag=f"ot{i}", name=f"ot{i}") for i in range(6)]
    wv_pool = ctx.enter_context(tc.tile_pool(name="wv", bufs=1))
    WV = [wv_pool.tile([128, C], BF16, tag=f"wv{i}", name=f"wv{i}") for i in range(6)]
    wp_pool = ctx.enter_context(tc.tile_pool(name="wp", bufs=1))
    WP = [wp_pool.tile([128, C], BF16, tag=f"wp{i}", name=f"wp{i}") for i in range(6)]

    # ---- DMA schedule (3 independent rings: sync, scalar, gpsimd) ----
    def dma_wqk(eng, pr, which):
        src = bass.AP(w_attn.tensor, which * C + pr * 128,
                      [[3 * C, 128], [3 * C * 128, 6], [1, 128]]).bitcast(BF16)
        dst = bass.AP(WQKp[pr].tensor, WQKp[pr].offset + which * 128,
                      [WQKp[pr].ap[0], [256, 6], [1, 128]])
        eng.dma_start(dst, src)

    # x row tiles split across BOTH hwdge rings (each ring sustains ~80GB/s,
    # so one ring alone paces the transposes at ~2.5us/tile): even tiles on
    # sync, odd tiles on scalar with WV wedged between so V projection can
    # start right as the transposes finish. Q/K weight slices follow behind.
    x_pool = ctx.enter_context(tc.tile_pool(name="xs", bufs=1))
    XM = [x_pool.tile([128, C], BF16, tag=f"x{mt}", name=f"xm{mt}")
          for mt in range(8)]

    def dma_x(eng, mt):
        eng.dma_start(XM[mt][:], x[mt * 128:(mt + 1) * 128, :].bitcast(BF16))

    for mt in (0, 2, 4, 6):
        dma_x(nc.sync, mt)
    for pr in range(6):
        dma_wqk(nc.sync, pr, 0)
    # scalar ring (ACT hwdge; idle until the first exp ~30us in)
    dma_x(nc.scalar, 1)
    dma_x(nc.scalar, 3)
    for k in range(6):
        nc.scalar.dma_start(WV[k][:],
                            w_attn[k * 128:(k + 1) * 128, 2 * C:].bitcast(BF16))
    dma_x(nc.scalar, 5)
    dma_x(nc.scalar, 7)
    for pr in range(6):
        dma_wqk(nc.scalar, pr, 1)
    # gpsimd (software DGE, ~25ns issue): biases + late-needed WP
    # biases: b_attn[0:768] as [128, 6] (col p = b_q[p*128:(p+1)*128]); K bias
    # is never added (softmax-invariant given Q carries b_q).
    bqk = const.tile([128, 6], F32, tag="bqk", name="bqk")
    nc.gpsimd.dma_start(bqk[:, :], b_attn[0:768].rearrange("(n p) -> p n", p=128))
    bv_row = const.tile([1, C], F32, tag="bv_row", name="bv_row")
    nc.gpsimd.dma_start(bv_row[:], b_attn[1536:2304].rearrange("(o f) -> o f", o=1))
    bv = const.tile([128, C], F32, tag="bv", name="bv")
    nc.gpsimd.partition_broadcast(bv[:], bv_row[:1, :])
    bp_row = const.tile([1, C], F32, tag="bp_row", name="bp_row")
    nc.gpsimd.dma_start(bp_row[:], b_proj[:].rearrange("(o f) -> o f", o=1))
    bp = const.tile([128, C], F32, tag="bp", name="bp")
    nc.gpsimd.partition_broadcast(bp[:], bp_row[:1, :])
    for k in range(6):
        nc.gpsimd.dma_start(WP[k][:], w_proj[k * 128:(k + 1) * 128, :].bitcast(BF16))
    for i in range(8):
        ones_ap = bass.AP(V[i].tensor, V[i].offset + 64, [V[i].ap[0], [192, 6], [1, 64]])
        nc.gpsimd.memset(ones_ap, 1.0)

    # PE warm-up during the initial DMA wait: dependency-free junk transposes
    # ride the p-state ramp so real work starts at full clock
    warm = psum.tile([128, 512], BF16, tag="ot", name="warm")
    for _ in range(12):
        nc.tensor.transpose(warm[:, 0:128], ident[:], ident[:])

    # ---- x transpose per row tile (PE) + strided eviction into XT ----
    def issue_T(mt):
        tp = psum.tile([128, 1024], BF16, tag="st", name="tp")
        for kc in range(6):
            nc.tensor.transpose(tp[:, kc * 128:(kc + 1) * 128],
                                XM[mt][:, kc * 128:(kc + 1) * 128], ident[:])
        src = bass.AP(tp.tensor, tp.offset, [tp.ap[0], [128, 6], [1, 128]])
        dst = bass.AP(XT.tensor, XT.offset + mt * 128,
                      [XT.ap[0], [1024, 6], [1, 128]])
        nc.vector.tensor_copy(dst, src)

    # ---- V projection per row tile (generator form so the later tiles can
    # weave into pair 0's attention; tiles 0-3 cover every key pair 0's first
    # query-half needs) ----
    def gen_V(mts):
        for mt in mts:
            for off, w in ((0, 512), (512, 256)):
                pv = psum.tile([128, 512], F32, tag="pq", name="pv")
                for kc in range(6):
                    nc.tensor.matmul(
                        pv[:, :w],
                        XT[:, kc * 1024 + mt * 128:kc * 1024 + mt * 128 + 128],
                        WV[kc][:, off:off + w],
                        start=(kc == 0), stop=(kc == 5))
                    if kc % 2 == 1:
                        yield
                # scatter natural cols [off, off+w) into the pair-group layout
                a = w // 128
                p0 = off // 128
                for par in range(2):
                    src_ap = bass.AP(pv.tensor, pv.offset + par * 64,
                                     [pv.ap[0], [128, a], [1, 64]])
                    dst_ap = bass.AP(V[mt].tensor,
                                     V[mt].offset + p0 * 192 + par * 128,
                                     [V[mt].ap[0], [192, a], [1, 64]])
                    bv_ap = bass.AP(bv.tensor, bv.offset + off + par * 64,
                                    [bv.ap[0], [128, a], [1, 64]])
                    nc.vector.tensor_add(dst_ap, src_ap, bv_ap)
                yield

    # ---- QK projection generator (woven into the previous pair's attention
    # and the V-projection loop). mc-major so mc=0 groups only need the first
    # query-half of XT. Drops the K bias. ----
    def gen_qkproj(pr):
        for mc in (0, 1):
            for which in (0, 1):  # 0 = Q, 1 = K
                pq = psum.tile([128, 512], F32, tag="pq", name="pq")
                for kc in range(6):
                    nc.tensor.matmul(
                        pq[:],
                        WQKp[pr][:, kc * 256 + which * 128:kc * 256 + which * 128 + 128],
                        XT[:, kc * 1024 + mc * 512:kc * 1024 + (mc + 1) * 512],
                        start=(kc == 0), stop=(kc == 5))
                    if kc % 2 == 1:
                        yield
                if which == 0:
                    nc.vector.scalar_tensor_tensor(
                        QT[pr][:, mc * 512:(mc + 1) * 512], pq[:],
                        bqk[:, pr:pr + 1], bv[:, 0:512], op0=ADD, op1=BYPASS)
                else:
                    nc.vector.tensor_copy(KT[pr][:, mc * 512:(mc + 1) * 512],
                                          pq[:])
                yield

    def drain(g, n):
        for _ in range(n):
            if g is None:
                return
            try:
                next(g)
            except StopIteration:
                return

    def chain(*gens):
        for g in gens:
            yield from g

    for mt in range(8):
        issue_T(mt)
    for _ in gen_V(range(4)):
        pass
    for _ in gen_qkproj(0):
        pass

    y_pool = ctx.enter_context(tc.tile_pool(name="ysb", bufs=3))

    def issue_outproj(mt, split_dma=False):
        ysb = y_pool.tile([128, C], F32, tag="y", name="ysb")
        for off, w in ((0, 512), (512, 256)):
            py = psum.tile([128, 512], F32, tag="pq", name="py")
            for kc in range(6):
                nc.tensor.matmul(
                    py[:, :w], OT[kc][:, mt * 128:(mt + 1) * 128],
                    WP[kc][:, off:off + w],
                    start=(kc == 0), stop=(kc == 5))
                if kc % 3 == 2:
                    yield
            nc.vector.tensor_add(ysb[:, off:off + w], py[:, :w],
                                 bp[:, off:off + w])
        if split_dma == 4:  # last tile: quarter the end-of-kernel DMA drain
            for s, eng in enumerate((nc.sync, nc.scalar, nc.sync, nc.scalar)):
                eng.dma_start(y[mt * 128 + s * 32:mt * 128 + (s + 1) * 32, :],
                              ysb[s * 32:(s + 1) * 32, :])
        elif split_dma:  # halve the DMA drain across both rings
            nc.sync.dma_start(y[mt * 128:mt * 128 + 64, :], ysb[0:64, :])
            nc.scalar.dma_start(y[mt * 128 + 64:(mt + 1) * 128, :], ysb[64:128, :])
        else:
            nc.scalar.dma_start(y[mt * 128:(mt + 1) * 128, :], ysb[:])
        yield

    # out-proj row tiles 0-3 only touch OT[.][:, 0:512]. During pair 5's
    # first query-half, kc 0..4 of tile 0 can already accumulate (they don't
    # read OT[5]); the rest follows once that half is normalized.
    woven_py = []
    woven_py4 = []

    def gen_outproj_ci0():
        for off, w in ((0, 512), (512, 256)):
            py = psum.tile([128, 512], F32, tag="pq", name="py")
            woven_py.append((py, off, w))
            for kc in range(5):
                nc.tensor.matmul(py[:, :w], OT[kc][:, 0:128],
                                 WP[kc][:, off:off + w],
                                 start=(kc == 0), stop=False)
                yield

    def gen_outproj_ci1():
        ysb = y_pool.tile([128, C], F32, tag="y", name="ysb")
        for py, off, w in woven_py:
            nc.tensor.matmul(py[:, :w], OT[5][:, 0:128], WP[5][:, off:off + w],
                             start=False, stop=True)
            nc.vector.tensor_add(ysb[:, off:off + w], py[:, :w],
                                 bp[:, off:off + w])
            yield
        nc.scalar.dma_start(y[0:128, :], ysb[:])
        for mt in range(1, 4):
            yield from issue_outproj(mt)
        # open row-tile 4's accumulation too: kc 0..4 never read OT[5]
        for off, w in ((0, 512), (512, 256)):
            py = psum.tile([128, 512], F32, tag="pq", name="py")
            woven_py4.append((py, off, w))
            for kc in range(5):
                nc.tensor.matmul(py[:, :w], OT[kc][:, 512:640],
                                 WP[kc][:, off:off + w],
                                 start=(kc == 0), stop=False)
                yield

    # ---- attention, head pairs, query-halved; software-pipelined jb loop ----
    pt_pool = ctx.enter_context(tc.tile_pool(name="ptp", bufs=3))
    nrm_pool = ctx.enter_context(tc.tile_pool(name="nrm", bufs=4))

    def attention(pr, gens):
        QTt, KTt = QT[pr], KT[pr]
        for ci in (0, 1):
            g = gens[ci]
            qlo = ci * 512
            njb = 4 + 4 * ci
            otp = [psum.tile([128, 512], F32, tag="ot", name="otp")
                   for _ in range(2)]
            info = [None] * njb

            def issue_ST(jb):
                jlo = jb * 128
                s = max(jlo, qlo)
                w = qlo + 512 - s
                st = psum.tile([128, 1024], F32, tag="st", name="st")
                for par in range(2):
                    nc.tensor.matmul(st[:, par * 512:par * 512 + w],
                                     KTt[par * 64:par * 64 + 64, jlo:jlo + 128],
                                     QTt[par * 64:par * 64 + 64, s:s + w],
                                     start=True, stop=True)
                ptp = pt_pool.tile([128, 1024], F16, tag="pt", name="ptp")
                if w == 512:
                    nc.scalar.activation(ptp[:, 0:1024], st[:, 0:1024], EXP,
                                         scale=SCALE)
                else:
                    for par in range(2):
                        nc.scalar.activation(ptp[:, par * 512:par * 512 + w],
                                             st[:, par * 512:par * 512 + w],
                                             EXP, scale=SCALE)
                if s == jlo:  # diagonal block sits at chunk col 0
                    for par in range(2):
                        diag = ptp[:, par * 512:par * 512 + 128]
                        nc.vector.tensor_mul(diag, diag, mask[:])
                return (ptp, s, w)

            def issue_O(jb):
                ptp, s, w = info[jb]
                for par in range(2):
                    nc.tensor.matmul(
                        otp[par][:, s - qlo:s - qlo + w],
                        V[jb][:, pr * 192 + par * 64:pr * 192 + par * 64 + 128],
                        ptp[:, par * 512:par * 512 + w],
                        start=(jb == 0), stop=(jb == njb - 1))

            if ci == 1 and gens[0] is not gens[1]:
                drain(gens[0], 99)
            info[0] = issue_ST(0)
            for jb in range(1, njb):
                info[jb] = issue_ST(jb)
                drain(g, 3)
                issue_O(jb - 1)
            drain(g, 2)
            issue_O(njb - 1)
            # custom-DVE reciprocal needs base-partition-0 operands: par=0's
            # denominator (PSUM rows 64:128) must be shift-copied down first;
            # par=1's (rows 0:64) is read straight from PSUM.
            for par in range(2):
                o_rows = slice(0, 64) if par == 0 else slice(64, 128)
                recip = nrm_pool.tile([64, 512], F32, tag="recip", name="recip")
                if par == 0:
                    den = nrm_pool.tile([64, 512], F32, tag="den", name="den")
                    nc.vector.tensor_copy(den[:], otp[0][64:128, :])
                    nc.vector.reciprocal_approx_fast(recip[:], den[:])
                else:
                    nc.vector.reciprocal_approx_fast(recip[:], otp[1][0:64, :])
                nc.vector.tensor_mul(OT[pr][par * 64:(par + 1) * 64, qlo:qlo + 512],
                                     otp[par][o_rows, :], recip[:])
                drain(g, 2)

    for pr in range(6):
        if pr == 0:
            g = chain(gen_V(range(4, 8)), gen_qkproj(1))
            gens = {0: g, 1: g}
        elif pr < 5:
            g = gen_qkproj(pr + 1)
            gens = {0: g, 1: g}
        else:
            gens = {0: gen_outproj_ci0(), 1: gen_outproj_ci1()}
        attention(pr, gens)
        drain(gens[0], 99)
        drain(gens[1], 99)

    # ---- output projection tail ----
    if len(woven_py4) == 2:  # close row tile 4 (kc=5 + bias + writeback)
        ysb4 = y_pool.tile([128, C], F32, tag="y", name="ysb4")
        for py, off, w in woven_py4:
            nc.tensor.matmul(py[:, :w], OT[5][:, 512:640], WP[5][:, off:off + w],
                             start=False, stop=True)
            nc.vector.tensor_add(ysb4[:, off:off + w], py[:, :w],
                                 bp[:, off:off + w])
        nc.sync.dma_start(y[512:576, :], ysb4[0:64, :])
        nc.scalar.dma_start(y[576:640, :], ysb4[64:128, :])
        rest = range(5, 8)
    else:
        rest = range(4, 8)
    for mt in rest:
        for _ in issue_outproj(mt, split_dma=4 if mt == 7 else True):
            pass


_NC_CACHE = None


def _build():
    global _NC_CACHE
    if _NC_CACHE is not None:
        return _NC_CACHE
    nc = bacc.Bacc("TRN2", target_bir_lowering=False, debug=False,
                   num_devices=N_CORES)
    # x / w_attn / w_proj arrive as bf16 bit patterns (host-cast)
    x = nc.dram_tensor("x", [T, C], U16, kind="ExternalInput").ap()
    w_attn = nc.dram_tensor("w_attn", [C, 3 * C], U16, kind="ExternalInput").ap()
    b_attn = nc.dram_tensor("b_attn", [3 * C], F32, kind="ExternalInput").ap()
    w_proj = nc.dram_tensor("w_proj", [C, C], U16, kind="ExternalInput").ap()
    b_proj = nc.dram_tensor("b_proj", [C], F32, kind="ExternalInput").ap()
    y = nc.dram_tensor("y", [T, C], F32, kind="ExternalOutput").ap()
    with tile.TileContext(nc) as tc, ExitStack() as ctx:
        _body(ctx, tc, y, x, w_attn, b_attn, w_proj, b_proj)
    nc.compile()
    _NC_CACHE = nc
    return nc


def _bf16_bits(a):
    """Round-to-nearest-even fp32 -> bf16, returned as uint16 bit patterns."""
    u = np.ascontiguousarray(np.asarray(a, np.float32)).view(np.uint32)
    lsb = (u >> 16) & 1
    return ((u + 0x7FFF + lsb) >> 16).astype(np.uint16)


def _run(inputs, trace=False):
    nc = _build()
    x = np.asarray(inputs["x"], dtype=np.float32)
    shared = {
        "w_attn": _bf16_bits(inputs["w_attn"]),
        "b_attn": np.ascontiguousarray(np.asarray(inputs["b_attn"], np.float32)),
        "w_proj": _bf16_bits(inputs["w_proj"]),
        "b_proj": np.ascontiguousarray(np.asarray(inputs["b_proj"], np.float32)),
    }
    in_maps = [dict(x=_bf16_bits(x[b]), **shared) for b in range(N_CORES)]
    res = run_bass_kernel_spmd(nc, in_maps, core_ids=list(range(N_CORES)),
                               trace=trace)
    out = np.stack([res.results[b]["y"] for b in range(N_CORES)], axis=0)
    return out.astype(np.float32), res


def kernel(**inputs):
    out, _ = _run(inputs, trace=False)
    return out


# revision 47
# speedup vs baseline: 1.0193x; 1.0193x over previous
"""Causal self-attention (B=8, T=1024, C=768, NH=12) on 8 TRN2 NeuronCores.

Sharding: pure data-parallel over batch -- one batch element per core, weights
replicated. No collectives needed.

Per-core algorithm (v5):
  - x, w_attn, w_proj are cast to bf16 on the host (rel err ~4e-3 vs the 2e-2
    gate), halving startup HBM traffic.
  - x^T lands in SBUF directly via XBAR DMA-transpose (no PE transposes, no
    PSUM->SBUF evictions). DMA issue cost (~0.6us/transfer on the issuing
    queue) is spread across engines: sync carries x^T, scalar carries
    WV/WQK + the y writebacks, gpsimd (software DGE, ~25ns issue) carries
    biases + WP.
  - V projection per row tile (+bias scatter into the pair-grouped V_aug fp16
    layout with appended ones-columns).
  - QK projection per head-pair is WOVEN into the previous pair's attention
    so PE work hides ACT-exp latency. The K bias is dropped entirely (softmax
    over keys is invariant to it once Q carries its own bias): K eviction is
    a plain cast-copy, Q eviction adds b_q.
  - Attention per pair, query-halved (ci-split): O/denominator accumulate in
    2 PSUM banks per half, freeing banks for the woven projections.
    ST = K^T-block @ Q (fp16, keys on partitions), one exp per chunk (both
    heads share a 2-bank ST tile), causal mask as a 0/1 fp16 multiply on the
    diagonal 128-block, O^T += V_aug-block @ P with ones-columns producing
    the softmax denominator rows for free. P is fp16 (logits*scale ~
    N(0, 0.31)). Normalization: reciprocal from PSUM (base-0 partitions) or
    via one shift-copy; one multiply into bf16 OT.
  - y = OT.T @ Wp + bp. Row tiles 0-3 only need the first query-half of
    pair 5's OT, so they are woven into pair 5's second attention half.
"""
import numpy as np
from contextlib import ExitStack

import concourse.bass as bass
import concourse.tile as tile
from concourse import bacc, mybir
from concourse.bass_utils import run_bass_kernel_spmd
from concourse.masks import make_identity, make_upper_triangular

T, C, NH, HD = 1024, 768, 12, 64
N_CORES = 8
SCALE = 1.0 / 8.0  # 1/sqrt(HD)

F32 = mybir.dt.float32
F16 = mybir.dt.float16
BF16 = mybir.dt.bfloat16
U16 = mybir.dt.uint16
EXP = mybir.ActivationFunctionType.Exp
ADD = mybir.AluOpType.add
BYPASS = mybir.AluOpType.bypass


def _body(ctx, tc, y, x, w_attn, b_attn, w_proj, b_proj):
    nc = tc.nc

    const = ctx.enter_context(tc.tile_pool(name="const", bufs=1))
    # PSUM: st 2x[128,1024]f32 (4 banks: ST tiles), ot 2x[128,512]f32 (2
    # banks: O/denominator, one query-half at a time), pq 2x[128,512]f32 (2
    # banks: V/QK/out projections). 16KB/partition total.
    psum = ctx.enter_context(tc.tile_pool(name="psum", bufs=2, space="PSUM"))

    # ---- constants (gpsimd; also used to warm the PE p-state) ----
    ident = const.tile([128, 128], BF16, tag="ident", name="ident")
    make_identity(nc, ident[:])
    # mask[j, i] = 1 if i >= j else 0 (keep key j for query i when i >= j)
    mask = const.tile([128, 128], F16, tag="mask", name="mask")
    make_upper_triangular(nc, mask[:], val=1.0, diag=True)

    # ---- persistent tensors ----
    qk_pool = ctx.enter_context(tc.tile_pool(name="qk", bufs=1))
    QT = [qk_pool.tile([128, T], F16, tag=f"qt{i}", name=f"qt{i}") for i in range(6)]
    KT = [qk_pool.tile([128, T], F16, tag=f"kt{i}", name=f"kt{i}") for i in range(6)]
    # V_aug: 6 pair-groups of 192 cols: [V_{2p} (64) | ones (64) | V_{2p+1} (64)]
    # -> per-head lhsT is the contiguous 128-col slice [p*192 + (h%2)*64, +128):
    #    even head: [V_h | ones] (PSUM rows 0:64 = O^T, 64:128 = denom)
    #    odd head:  [ones | V_h] (rows flipped)
    v_pool = ctx.enter_context(tc.tile_pool(name="v", bufs=1))
    V = [v_pool.tile([128, 1152], F16, tag=f"v{i}", name=f"v{i}") for i in range(8)]

    xw_pool = ctx.enter_context(tc.tile_pool(name="xw", bufs=1))
    # XT[c_part, kc*1024 + t]: transposed x, channel-major
    XT = xw_pool.tile([128, 6 * T], BF16, tag="xt", name="xt")
    # WQKp[pr][c_part, kc*256 + which*128 + d]: per-pair column slices of
    # w_attn, so pair pr's weights stream in just before its woven projection
    WQKp = [xw_pool.tile([128, 1536], BF16, tag=f"w{i}", name=f"w{i}")
            for i in range(6)]
    ot_pool = ctx.enter_context(tc.tile_pool(name="ot", bufs=1))
    OT = [ot_pool.tile([128, T], BF16, tag=f"ot{i}", name=f"ot{i}") for i in range(6)]
    wv_pool = ctx.enter_context(tc.tile_pool(name="wv", bufs=1))
    WV = [wv_pool.tile([128, C], BF16, tag=f"wv{i}", name=f"wv{i}") for i in range(6)]
    wp_pool = ctx.enter_context(tc.tile_pool(name="wp", bufs=1))
    WP = [wp_pool.tile([128, C], BF16, tag=f"wp{i}", name=f"wp{i}") for i in range(6)]

    # ---- DMA schedule (3 independent rings: sync, scalar, gpsimd) ----
    def dma_wqk(eng, pr, which):
        src = bass.AP(w_attn.tensor, which * C + pr * 128,
                      [[3 * C, 128], [3 * C * 128, 6], [1, 128]]).bitcast(BF16)
        dst = bass.AP(WQKp[pr].tensor, WQKp[pr].offset + which * 128,
                      [WQKp[pr].ap[0], [256, 6], [1, 128]])
        eng.dma_start(dst, src)

    # x row tiles split across BOTH hwdge rings (each ring sustains ~80GB/s,
    # so one ring alone paces the transposes at ~2.5us/tile): even tiles on
    # sync, odd tiles on scalar with WV wedged between so V projection can
    # start right as the transposes finish. Q/K weight slices follow behind.
    x_pool = ctx.enter_context(tc.tile_pool(name="xs", bufs=1))
    XM = [x_pool.tile([128, C], BF16, tag=f"x{mt}", name=f"xm{mt}")
          for mt in range(8)]

    def dma_x(eng, mt):
        eng.dma_start(XM[mt][:], x[mt * 128:(mt + 1) * 128, :].bitcast(BF16))

    for mt in (0, 2, 4, 6):
        dma_x(nc.sync, mt)
    for pr in range(6):
        dma_wqk(nc.sync, pr, 0)
    # scalar ring (ACT hwdge; idle until the first exp ~30us in)
    dma_x(nc.scalar, 1)
    dma_x(nc.scalar, 3)
    for k in range(6):
        nc.scalar.dma_start(WV[k][:],
                            w_attn[k * 128:(k + 1) * 128, 2 * C:].bitcast(BF16))
    dma_x(nc.scalar, 5)
    dma_x(nc.scalar, 7)
    for pr in range(6):
        dma_wqk(nc.scalar, pr, 1)
    # gpsimd (software DGE, ~25ns issue): biases + late-needed WP
    # biases: b_attn[0:768] as [128, 6] (col p = b_q[p*128:(p+1)*128]); K bias
    # is never added (softmax-invariant given Q carries b_q).
    bqk = const.tile([128, 6], F32, tag="bqk", name="bqk")
    nc.gpsimd.dma_start(bqk[:, :], b_attn[0:768].rearrange("(n p) -> p n", p=128))
    bv_row = const.tile([1, C], F32, tag="bv_row", name="bv_row")
    nc.gpsimd.dma_start(bv_row[:], b_attn[1536:2304].rearrange("(o f) -> o f", o=1))
    bv = const.tile([128, C], F32, tag="bv", name="bv")
    nc.gpsimd.partition_broadcast(bv[:], bv_row[:1, :])
    bp_row = const.tile([1, C], F32, tag="bp_row", name="bp_row")
    nc.gpsimd.dma_start(bp_row[:], b_proj[:].rearrange("(o f) -> o f", o=1))
    bp = const.tile([128, C], F32, tag="bp", name="bp")
    nc.gpsimd.partition_broadcast(bp[:], bp_row[:1, :])
    for k in range(6):
        nc.gpsimd.dma_start(WP[k][:], w_proj[k * 128:(k + 1) * 128, :].bitcast(BF16))
    for i in range(8):
        ones_ap = bass.AP(V[i].tensor, V[i].offset + 64, [V[i].ap[0], [192, 6], [1, 64]])
        nc.gpsimd.memset(ones_ap, 1.0)

    # PE warm-up during the initial DMA wait: dependency-free junk transposes
    # ride the p-state ramp so real work starts at full clock
    warm = psum.tile([128, 512], BF16, tag="ot", name="warm")
    for _ in range(12):
        nc.tensor.transpose(warm[:, 0:128], ident[:], ident[:])

    # ---- x transpose per row tile (PE) + strided eviction into XT ----
    def issue_T(mt):
        tp = psum.tile([128, 1024], BF16, tag="st", name="tp")
        for kc in range(6):
            nc.tensor.transpose(tp[:, kc * 128:(kc + 1) * 128],
                                XM[mt][:, kc * 128:(kc + 1) * 128], ident[:])
        src = bass.AP(tp.tensor, tp.offset, [tp.ap[0], [128, 6], [1, 128]])
        dst = bass.AP(XT.tensor, XT.offset + mt * 128,
                      [XT.ap[0], [1024, 6], [1, 128]])
        nc.vector.tensor_copy(dst, src)

    # ---- V projection per row tile (generator form so the later tiles can
    # weave into pair 0's attention; tiles 0-3 cover every key pair 0's first
    # query-half needs) ----
    def gen_V(mts):
        for mt in mts:
            for off, w in ((0, 512), (512, 256)):
                pv = psum.tile([128, 512], F32, tag="pq", name="pv")
                for kc in range(6):
                    nc.tensor.matmul(
                        pv[:, :w],
                        XT[:, kc * 1024 + mt * 128:kc * 1024 + mt * 128 + 128],
                        WV[kc][:, off:off + w],
                        start=(kc == 0), stop=(kc == 5))
                    if kc % 2 == 1:
                        yield
                # scatter natural cols [off, off+w) into the pair-group layout
                a = w // 128
                p0 = off // 128
                for par in range(2):
                    src_ap = bass.AP(pv.tensor, pv.offset + par * 64,
                                     [pv.ap[0], [128, a], [1, 64]])
                    dst_ap = bass.AP(V[mt].tensor,
                                     V[mt].offset + p0 * 192 + par * 128,
                                     [V[mt].ap[0], [192, a], [1, 64]])
                    bv_ap = bass.AP(bv.tensor, bv.offset + off + par * 64,
                                    [bv.ap[0], [128, a], [1, 64]])
                    nc.vector.tensor_add(dst_ap, src_ap, bv_ap)
                yield

    # ---- QK projection generator (woven into the previous pair's attention
    # and the V-projection loop). mc-major so mc=0 groups only need the first
    # query-half of XT. Drops the K bias. ----
    def gen_qkproj(pr):
        for mc in (0, 1):
            for which in (0, 1):  # 0 = Q, 1 = K
                pq = psum.tile([128, 512], F32, tag="pq", name="pq")
                for kc in range(6):
                    nc.tensor.matmul(
                        pq[:],
                        WQKp[pr][:, kc * 256 + which * 128:kc * 256 + which * 128 + 128],
                        XT[:, kc * 1024 + mc * 512:kc * 1024 + (mc + 1) * 512],
                        start=(kc == 0), stop=(kc == 5))
                    if kc % 2 == 1:
                        yield
                if which == 0:
                    nc.vector.scalar_tensor_tensor(
                        QT[pr][:, mc * 512:(mc + 1) * 512], pq[:],
                        bqk[:, pr:pr + 1], bv[:, 0:512], op0=ADD, op1=BYPASS)
                else:
                    nc.vector.tensor_copy(KT[pr][:, mc * 512:(mc + 1) * 512],
                                          pq[:])
                yield

    def drain(g, n):
        for _ in range(n):
            if g is None:
                return
            try:
                next(g)
            except StopIteration:
                return

    def chain(*gens):
        for g in gens:
            yield from g

    for mt in range(8):
        issue_T(mt)
    for _ in gen_V(range(4)):
        pass
    for _ in gen_qkproj(0):
        pass

    y_pool = ctx.enter_context(tc.tile_pool(name="ysb", bufs=3))

    def issue_outproj(mt, split_dma=False):
        ysb = y_pool.tile([128, C], F32, tag="y", name="ysb")
        for off, w in ((0, 512), (512, 256)):
            py = psum.tile([128, 512], F32, tag="pq", name="py")
            for kc in range(6):
                nc.tensor.matmul(
                    py[:, :w], OT[kc][:, mt * 128:(mt + 1) * 128],
                    WP[kc][:, off:off + w],
                    start=(kc == 0), stop=(kc == 5))
                if kc % 3 == 2:
                    yield
            nc.vector.tensor_add(ysb[:, off:off + w], py[:, :w],
                                 bp[:, off:off + w])
        if split_dma == 4:  # last tile: quarter the end-of-kernel DMA drain
            for s, eng in enumerate((nc.sync, nc.scalar, nc.sync, nc.scalar)):
                eng.dma_start(y[mt * 128 + s * 32:mt * 128 + (s + 1) * 32, :],
                              ysb[s * 32:(s + 1) * 32, :])
        elif split_dma:  # halve the DMA drain across both rings
            nc.sync.dma_start(y[mt * 128:mt * 128 + 64, :], ysb[0:64, :])
            nc.scalar.dma_start(y[mt * 128 + 64:(mt + 1) * 128, :], ysb[64:128, :])
        else:
            nc.scalar.dma_start(y[mt * 128:(mt + 1) * 128, :], ysb[:])
        yield

    # out-proj row tiles 0-3 only touch OT[.][:, 0:512]. During pair 5's
    # first query-half, kc 0..4 of tile 0 can already accumulate (they don't
    # read OT[5]); the rest follows once that half is normalized.
    woven_py = []

    def gen_outproj_ci0():
        for off, w in ((0, 512), (512, 256)):
            py = psum.tile([128, 512], F32, tag="pq", name="py")
            woven_py.append((py, off, w))
            for kc in range(5):
                nc.tensor.matmul(py[:, :w], OT[kc][:, 0:128],
                                 WP[kc][:, off:off + w],
                                 start=(kc == 0), stop=False)
                yield

    def gen_outproj_ci1():
        ysb = y_pool.tile([128, C], F32, tag="y", name="ysb")
        for py, off, w in woven_py:
            nc.tensor.matmul(py[:, :w], OT[5][:, 0:128], WP[5][:, off:off + w],
                             start=False, stop=True)
            nc.vector.tensor_add(ysb[:, off:off + w], py[:, :w],
                                 bp[:, off:off + w])
            yield
        nc.scalar.dma_start(y[0:128, :], ysb[:])
        for mt in range(1, 4):
            yield from issue_outproj(mt)

    # ---- attention, head pairs, query-halved; software-pipelined jb loop ----
    pt_pool = ctx.enter_context(tc.tile_pool(name="ptp", bufs=3))
    nrm_pool = ctx.enter_context(tc.tile_pool(name="nrm", bufs=4))

    def attention(pr, gens):
        QTt, KTt = QT[pr], KT[pr]
        for ci in (0, 1):
            g = gens[ci]
            qlo = ci * 512
            njb = 4 + 4 * ci
            otp = [psum.tile([128, 512], F32, tag="ot", name="otp")
                   for _ in range(2)]
            info = [None] * njb

            def issue_ST(jb):
                jlo = jb * 128
                s = max(jlo, qlo)
                w = qlo + 512 - s
                st = psum.tile([128, 1024], F32, tag="st", name="st")
                for par in range(2):
                    nc.tensor.matmul(st[:, par * 512:par * 512 + w],
                                     KTt[par * 64:par * 64 + 64, jlo:jlo + 128],
                                     QTt[par * 64:par * 64 + 64, s:s + w],
                                     start=True, stop=True)
                ptp = pt_pool.tile([128, 1024], F16, tag="pt", name="ptp")
                if w == 512:
                    nc.scalar.activation(ptp[:, 0:1024], st[:, 0:1024], EXP,
                                         scale=SCALE)
                else:
                    for par in range(2):
                        nc.scalar.activation(ptp[:, par * 512:par * 512 + w],
                                             st[:, par * 512:par * 512 + w],
                                             EXP, scale=SCALE)
                if s == jlo:  # diagonal block sits at chunk col 0
                    for par in range(2):
                        diag = ptp[:, par * 512:par * 512 + 128]
                        nc.vector.tensor_mul(diag, diag, mask[:])
                return (ptp, s, w)

            def issue_O(jb):
                ptp, s, w = info[jb]
                for par in range(2):
                    nc.tensor.matmul(
                        otp[par][:, s - qlo:s - qlo + w],
                        V[jb][:, pr * 192 + par * 64:pr * 192 + par * 64 + 128],
                        ptp[:, par * 512:par * 512 + w],
                        start=(jb == 0), stop=(jb == njb - 1))

            if ci == 1 and gens[0] is not gens[1]:
                drain(gens[0], 99)
            info[0] = issue_ST(0)
            for jb in range(1, njb):
                info[jb] = issue_ST(jb)
                drain(g, 3 if ci else 2)
                issue_O(jb - 1)
            drain(g, 2)
            issue_O(njb - 1)
            # custom-DVE reciprocal needs base-partition-0 operands: par=0's
            # denominator (PSUM rows 64:128) must be shift-copied down first;
            # par=1's (rows 0:64) is read straight from PSUM.
            for par in range(2):
                o_rows = slice(0, 64) if par == 0 else slice(64, 128)
                recip = nrm_pool.tile([64, 512], F32, tag="recip", name="recip")
                if par == 0:
                    den = nrm_pool.tile([64, 512], F32, tag="den", name="den")
                    nc.vector.tensor_copy(den[:], otp[0][64:128, :])
                    nc.vector.reciprocal_approx_fast(recip[:], den[:])
                else:
                    nc.vector.reciprocal_approx_fast(recip[:], otp[1][0:64, :])
                nc.vector.tensor_mul(OT[pr][par * 64:(par + 1) * 64, qlo:qlo + 512],
                                     otp[par][o_rows, :], recip[:])
                drain(g, 2)

    for pr in range(6):
        if pr == 0:
            g = chain(gen_V(range(4, 8)), gen_qkproj(1))
            gens = {0: g, 1: g}
        elif pr < 5:
            g = gen_qkproj(pr + 1)
            gens = {0: g, 1: g}
        else:
            gens = {0: gen_outproj_ci0(), 1: gen_outproj_ci1()}
        attention(pr, gens)
        drain(gens[0], 99)
        drain(gens[1], 99)

    # ---- output projection tail (row tiles 4-7) ----
    for mt in range(4, 8):
        for _ in issue_outproj(mt, split_dma=4 if mt == 7 else True):
            pass


_NC_CACHE = None


def _build():
    global _NC_CACHE
    if _NC_CACHE is not None:
        return _NC_CACHE
    nc = bacc.Bacc("TRN2", target_bir_lowering=False, debug=False,
                   num_devices=N_CORES)
    # x / w_attn / w_proj arrive as bf16 bit patterns (host-cast)
    x = nc.dram_tensor("x", [T, C], U16, kind="ExternalInput").ap()
    w_attn = nc.dram_tensor("w_attn", [C, 3 * C], U16, kind="ExternalInput").ap()
    b_attn = nc.dram_tensor("b_attn", [3 * C], F32, kind="ExternalInput").ap()
    w_proj = nc.dram_tensor("w_proj", [C, C], U16, kind="ExternalInput").ap()
    b_proj = nc.dram_tensor("b_proj", [C], F32, kind="ExternalInput").ap()
    y = nc.dram_tensor("y", [T, C], F32, kind="ExternalOutput").ap()
    with tile.TileContext(nc) as tc, ExitStack() as ctx:
        _body(ctx, tc, y, x, w_attn, b_attn, w_proj, b_proj)
    nc.compile()
    _NC_CACHE = nc
    return nc


def _bf16_bits(a):
    """Round-to-nearest-even fp32 -> bf16, returned as uint16 bit patterns."""
    u = np.ascontiguousarray(np.asarray(a, np.float32)).view(np.uint32)
    lsb = (u >> 16) & 1
    return ((u + 0x7FFF + lsb) >> 16).astype(np.uint16)


def _run(inputs, trace=False):
    nc = _build()
    x = np.asarray(inputs["x"], dtype=np.float32)
    shared = {
        "w_attn": _bf16_bits(inputs["w_attn"]),
        "b_attn": np.ascontiguousarray(np.asarray(inputs["b_attn"], np.float32)),
        "w_proj": _bf16_bits(inputs["w_proj"]),
        "b_proj": np.ascontiguousarray(np.asarray(inputs["b_proj"], np.float32)),
    }
    in_maps = [dict(x=_bf16_bits(x[b]), **shared) for b in range(N_CORES)]
    res = run_bass_kernel_spmd(nc, in_maps, core_ids=list(range(N_CORES)),
                               trace=trace)
    out = np.stack([res.results[b]["y"] for b in range(N_CORES)], axis=0)
    return out.astype(np.float32), res


def kernel(**inputs):
    out, _ = _run(inputs, trace=False)
    return out
